# revision 74
# baseline (speedup 1.0000x reference)
"""Trainium2 Bass kernel for the Convpass-swin hypernet-fuse adapter module.

Data-parallel over batch: 32 samples -> 8 cores x 4 samples; small weights
replicated. All heavy matmuls run in bf16 (fp32 PSUM accumulate); the
tolerance budget (2e-2) dwarfs bf16 rounding (~2e-3).

Per-core dataflow (R = 4*28*28 = 3136 rows, C=768, D=EMB=64):
  1. x is transposed and bf16-cast on the host into chunk-major layout
     xtp[q*128+p, kt*392+n] (8 chunks of 392 rows). DMA order: wstk, all 8
     x chunks (just-in-time for the stacked matmuls), then hwt in 6 pieces
     so the hypernet matmuls stream right behind their own DMA.
  2. Per chunk: 6 K-tile matmuls -> PSUM [128, 392]; rows 0:64 ACT
     Relu(+b1, accum_out) for the meta path, rows 64:128 ACT
     Gelu_apprx_sigmoid(+down_b) = exact qgelu -> bf16 s1 in a zero-padded
     [128, 4, 30, 30] buffer; DVE duplicates it one column shifted on
     partitions 64:128 so conv taps (dh,0),(dh,1) fuse into K=128 matmuls.
  3. prompt accumulates in PSUM via 2 matmuls over mha column halves
     (w2.T/784 stationary); fused = prompt + b2 + layer_emb written twice
     into a tiny [128, 8] block-diagonal stationary (two parity chunks).
  4. Hypernet: one K=128 bf16 matmul per [128,512] weight tile -> PSUM
     [8, 512]; copies (DVE/ACT/GpSimd rotating) -> bf16 staging [8, 6144]
     per 12-tile group, bounced via DRAM scratch into conv-weight layout.
     Fetches for samples 0/1 are split by dh and prefetched as soon as the
     covering groups are written.
  5. Conv per (sample, 14-row half): 3 paired K=128 + 3 single K=64 bf16
     matmuls stream FLAT N=418 windows of the padded s1 (garbage lands in
     pad columns of PSUM [64, 420]); ACT qgelu reads the valid columns
     strided -> yg bf16 [65, R] (row 64 = ones).
  6. Up-projection in 25 M=128 row tiles across the whole yg (tiles may
     span samples): 2 matmuls (N=512+256) -> PSUM [128, 768], one rotating
     copy -> bf16 staging, output DMA per 2 tiles.
"""

import sys

sys.path.insert(0, "/opt/trn_rl_repo")

import ml_dtypes
import numpy as np

import concourse.bass as bass
import concourse.tile as tile
from concourse import bacc, mybir
from concourse.bass_utils import run_bass_kernel_spmd

F32 = mybir.dt.float32
BF = mybir.dt.bfloat16
AF = mybir.ActivationFunctionType
OP = mybir.AluOpType
BF16 = ml_dtypes.bfloat16

B, H, W, C, D, EMB = 32, 28, 28, 768, 64, 64
NCORES = 8
BL = B // NCORES            # samples per core
R = BL * H * W              # 3136 rows per core
HP, WP = H + 2, W + 2       # padded 30x30
JTOT = D * D * 9            # 36864 hypernet outputs per sample
NCH = JTOT // 512           # 72 chunks of 512
NHT = NCH // 2              # 36 hypernet weight tiles [128, 512]
HTG = 12                    # hypernet tiles per group (= one kh tap row)
NHWP = 6                    # hwt DMA pieces (6 tiles each)
NB = 392                    # half-sample chunk (14 rows of 28)
NQ = R // NB                # 8 stacked chunks per core
NF = 418                    # flat conv window length (13*30 + 28)
NUT = (R + 127) // 128      # 25 up-projection row tiles

TRACE = False               # set True (e.g. from test.py) to capture a profile
LAST_EXEC_NS = None         # filled from the profile when TRACE is on

_cached = {}

# f32 whose bit pattern is two bf16 1.0s, for memsets on bf16 tiles
_ONES_BF16_PAIR = float(
    np.frombuffer(np.array([0x3F803F80], dtype=np.uint32).tobytes(), dtype=np.float32)[0]
)


def _build_program():
    nc = bacc.Bacc("TRN2", target_bir_lowering=False, debug=False)

    xtp = nc.declare_dram_parameter("xtp", [128, NQ * 6 * NB], BF, isOutput=False).ap()
    wstk = nc.declare_dram_parameter("wstk", [128, C], BF, isOutput=False).ap()
    # packed small biases: col 0 = meta_b1, col 1 = down_b, col 2 = b2+emb
    csm = nc.declare_dram_parameter("csm", [64, 3], F32, isOutput=False).ap()
    w2t = nc.declare_dram_parameter("w2t", [64, 64], BF, isOutput=False).ap()
    hwt = nc.declare_dram_parameter("hwt", [128, NHT * 512], BF, isOutput=False).ap()
    hbp2 = nc.declare_dram_parameter("hbp2", [128, 192], BF, isOutput=False).ap()
    hbp3 = nc.declare_dram_parameter("hbp3", [64, 192], BF, isOutput=False).ap()
    upw = nc.declare_dram_parameter("upw", [65, C], BF, isOutput=False).ap()
    out = nc.declare_dram_parameter("out", [R, C], BF, isOutput=True).ap()

    with tile.TileContext(nc) as tc, \
         tc.tile_pool(name="consts", bufs=1) as cpool, \
         tc.tile_pool(name="xin", bufs=3) as xinpool, \
         tc.tile_pool(name="work", bufs=2) as wpool, \
         tc.tile_pool(name="cwsb", bufs=3) as cwsbpool, \
         tc.tile_pool(name="cwtp", bufs=4) as cwtpool, \
         tc.tile_pool(name="outp", bufs=2) as outpool, \
         tc.tile_pool(name="dram", bufs=1, space="DRAM") as dpool:

        # ---------- constants / standing buffers ----------
        # DMA issue order is the priority order on the Sync queue: wstk
        # first (every chunk matmul needs it), then the x chunks stream
        # just-in-time inside the loop, then hwt, then later-phase consts.
        wstk_sb = cpool.tile([128, 768], BF, tag="wstk")
        nc.sync.dma_start(out=wstk_sb[:], in_=wstk)

        s1pad = cpool.tile([128, BL * HP * WP], BF, tag="s1pad")
        nc.gpsimd.memset(s1pad[:].bitcast(F32), 0.0)
        mha_sb = cpool.tile([64, NQ], F32, tag="mha")
        fused_sb = cpool.tile([128, 2 * BL], BF, tag="fused")
        yg_sb = cpool.tile([65, R], BF, tag="yg")
        nc.vector.memset(yg_sb[64:65, :].bitcast(F32), _ONES_BF16_PAIR)
        hwt_sb = cpool.tile([128, NHT * 512], BF, tag="hwt")

        # preload both ACT function tables during the DMA-bound window so
        # the first real Relu / qgelu doesn't eat a ~1.3us table load
        ascr = cpool.tile([64, 1], F32, tag="ascr")
        nc.vector.memset(ascr[:], 0.0)
        ascr2 = cpool.tile([64, 1], F32, tag="ascr2")
        nc.scalar.activation(ascr2[:], ascr[:], AF.Relu)
        nc.scalar.activation(ascr2[:], ascr[:], AF.Gelu_apprx_sigmoid)

        s1v = s1pad[:].rearrange("p (b h w) -> p b h w", b=BL, h=HP, w=WP)
        s1f = s1pad[:]                     # flat [128, BL*900] view for conv

        # ---------- phase A: stacked meta1+down over 8 chunks, prompt ----------
        with tc.tile_pool(name="stkps", bufs=5, space="PSUM") as stkpool, \
             tc.tile_pool(name="auxps", bufs=1, space="PSUM") as auxpool:
            # PE clock warms up over ~5us of activity; burn the DMA-wait
            # window on junk matmuls so the first real chunk streams at
            # full rate instead of ~60% clock
            warm = auxpool.tile([128, 512], F32, tag="warm", name="warm")
            for _ in range(6):
                nc.tensor.matmul(
                    warm[:], lhsT=wstk_sb[:, 0:128], rhs=wstk_sb[:, 0:512],
                    start=True, stop=True, skip_group_check=True,
                )

            xq2 = None
            u = 0
            for q in range(NQ):
                b, hc = divmod(q, 2)
                if q < 2:
                    # first two chunks individually: lowest time-to-first-
                    # matmul while the queues are still cold
                    u = 0
                    xq2 = xinpool.tile([128, 6 * NB], BF, tag="xq1")
                    nc.sync.dma_start(
                        out=xq2[:], in_=xtp[:, q * 2352:(q + 1) * 2352]
                    )
                elif q % 2 == 0:
                    # then chunk pairs: 9408B/partition descriptors run
                    # ~30% faster per queue than single-chunk 4704B ones
                    u = 0
                    xq2 = xinpool.tile([128, 2 * 6 * NB], BF, tag="xq")
                    nc.sync.dma_start(
                        out=xq2[:],
                        in_=xtp[:, q * 2352:(q + 2) * 2352],
                    )
                else:
                    u = 1
                if q == 0:
                    csm_sb = cpool.tile([64, 3], F32, tag="csm")
                    nc.sync.dma_start(out=csm_sb[:], in_=csm)
                    brelu_sb = csm_sb[:, 0:1]
                    dwb_sb = csm_sb[:, 1:2]
                    fb_sb = csm_sb[:, 2:3]
                ps = stkpool.tile([128, NB], F32, tag="stk", name="ps")
                for kt in range(6):
                    nc.tensor.matmul(
                        ps[:],
                        lhsT=wstk_sb[:, kt * 128:(kt + 1) * 128],
                        rhs=xq2[:, u * 2352 + kt * NB:u * 2352 + (kt + 1) * NB],
                        start=(kt == 0),
                        stop=(kt == 5),
                    )
                hsc = wpool.tile([64, NB], BF, tag="hsc", name="hsc")
                ps3 = ps[64:128, :].rearrange("p (h w) -> p h w", h=14, w=W)
                h0 = hc * 14 + 1

                def relu_act():
                    nc.scalar.activation(
                        hsc[:], ps[0:64, :], AF.Relu,
                        bias=brelu_sb,
                    )

                def gelu_act():
                    nc.scalar.activation(
                        s1v[0:64, b, h0:h0 + 14, 1:W + 1], ps3,
                        AF.Gelu_apprx_sigmoid, bias=dwb_sb,
                    )

                # row-sums for the prompt go through DVE (tensor_reduce) to
                # keep the ACT engine at 2 ops/chunk, matching the PE rate
                # (fp32 out: a bf16 reduce also ACCUMULATES in bf16, which
                # destroys 392-term sums)

                def red_op():
                    nc.vector.tensor_reduce(
                        out=mha_sb[:, q:q + 1], in_=hsc[:],
                        axis=mybir.AxisListType.X, op=OP.add,
                    )

                def shift_op():
                    nc.vector.tensor_copy(
                        out=s1v[64:128, b, h0:h0 + 14, 0:W],
                        in_=s1v[0:64, b, h0:h0 + 14, 1:W + 1],
                    )

                # last chunk: gelu first on ACT / shift first on DVE, so
                # the prompt chain (gelu-independent) finishes earliest
                relu_act(); gelu_act(); red_op(); shift_op()

            # w2t is small and needed at the prompt (~25us); issue it ahead
            # of the big hypernet stream
            w2t_sb = cpool.tile([64, 64], BF, tag="w2t")
            nc.sync.dma_start(out=w2t_sb[:], in_=w2t)
            # hypernet weights stream right after the x chunks; the hyper
            # matmuls chase the pieces as they land
            npc = NHT * 512 // NHWP
            for i in range(NHWP):
                nc.sync.dma_start(
                    out=hwt_sb[:, i * npc:(i + 1) * npc],
                    in_=hwt[:, i * npc:(i + 1) * npc],
                )
            # consts for the conv/up phases ride behind
            upw_sb = cpool.tile([65, C], BF, tag="upw")
            nc.sync.dma_start(out=upw_sb[:], in_=upw)
            hbp2_sb = cpool.tile([128, 192], BF, tag="hbp2")
            nc.sync.dma_start(out=hbp2_sb[:], in_=hbp2)
            hbp3_sb = cpool.tile([64, 192], BF, tag="hbp3")
            nc.sync.dma_start(out=hbp3_sb[:], in_=hbp3)

            # prompt: pp[o, b] = sum_q w2t[:, o] . mha[:, (b, hc)], in bf16
            # (fp32 matmuls take 4 PE passes and a slow weight load)
            mhab = cpool.tile([64, NQ], BF, tag="mhab")
            nc.vector.tensor_copy(out=mhab[:], in_=mha_sb[:])
            mhv = mhab[:].rearrange("p (b h) -> p b h", b=BL)
            pp = auxpool.tile([64, BL], F32, tag="pp", name="pp")
            nc.tensor.matmul(
                pp[:], lhsT=w2t_sb[:], rhs=mhv[:, :, 0], start=True, stop=False,
            )
            nc.tensor.matmul(
                pp[:], lhsT=w2t_sb[:], rhs=mhv[:, :, 1], start=False, stop=True,
            )
            # block-diagonal [128, 8] stationary: rows 0:64 x cols 0:4 for
            # the even parity chunk, rows 64:128 x cols 4:8 for the odd
            nc.vector.memset(fused_sb[:].bitcast(F32), 0.0)
            nc.scalar.activation(fused_sb[0:64, 0:BL], pp[:], AF.Identity, bias=fb_sb)
            nc.scalar.activation(
                fused_sb[64:128, BL:2 * BL], pp[:], AF.Identity, bias=fb_sb
            )

        # ---------- phase B: hypernet ----------
        def copy_rot(i, out_ap, in_ap):
            # gpsimd cannot read PSUM on TRN2, so rotate over DVE and ACT
            if i % 2 == 0:
                nc.vector.tensor_copy(out=out_ap, in_=in_ap)
            else:
                nc.scalar.activation(out_ap, in_ap, AF.Copy)

        # host packs tile k of group g with chunks (24g+k, 24g+12+k) on the
        # two K-halves, so each cw_sb partition row is a CONTIGUOUS 6144-
        # element j-range: the group write is one DMA of 12KB descriptors
        cw_dram = dpool.tile([BL, JTOT], BF, tag="cw")
        # conv weight fetch view: j = (dh, (dw, di), do)
        cwt4 = cw_dram[:].rearrange(
            "b (dh dwdi do) -> b dwdi dh do", dh=3, dwdi=3 * D, do=D
        )
        cwp_t, cws_t = {}, {}

        def fetch_alloc():
            # single-tap (dw=2) weights live in rows 0:64 of a [128, 192]
            # tile whose upper half is zeroed, so every conv matmul runs
            # K=128 with identical PE tile_size (no reconfig stalls)
            for b in range(BL):
                cwp_t[b] = cwtpool.tile([128, 192], BF, tag="cwp", name="cwp")
                cws_t[b] = cwtpool.tile([128, 192], BF, tag="cws", name="cws")
                # DVE, not gpsimd: gpsimd op dispatch is ~us-slow and these
                # zeroed halves gate the conv single-tap matmuls
                nc.vector.memset(cws_t[b][64:128, :].bitcast(F32), 0.0)

        def fetch_sample(b, eng):
            eng.dma_start(
                out=cwp_t[b][:].rearrange("p (dh do) -> p dh do", dh=3),
                in_=cwt4[b, 0:128],
            )
            eng.dma_start(
                out=cws_t[b][0:64, :].rearrange("p (dh do) -> p dh do", dh=3),
                in_=cwt4[b, 128:192],
            )

        def slice_s0(g, cw_sb):
            # sample 0's conv stationaries gather SBUF->SBUF per group
            # (group g = tap row dh=g), skipping the DRAM bounce entirely:
            # parity row 0 holds kwdi 0:96, row 1 holds kwdi 96:192
            b = 0
            row0 = cw_sb[b:b + 1, :].rearrange("q (p do) -> q p do", do=64)
            row1 = cw_sb[BL + b:BL + b + 1, :].rearrange(
                "q (p do) -> q p do", do=64
            )
            nc.sync.dma_start(
                out=cwp_t[b][0:96, g * 64:(g + 1) * 64], in_=row0
            )
            nc.sync.dma_start(
                out=cwp_t[b][96:128, g * 64:(g + 1) * 64], in_=row1[:, 0:32]
            )
            nc.sync.dma_start(
                out=cws_t[b][0:64, g * 64:(g + 1) * 64], in_=row1[:, 32:96]
            )

        with tc.tile_pool(name="cwps", bufs=3, space="PSUM") as cwpool:
            for g in range(NHT // HTG):
                cw_sb = cwsbpool.tile([2 * BL, HTG * 512], BF, tag="cwsb")
                for k2 in range(HTG // 2):
                    # two matmul outputs per 2-bank PSUM tile, ONE copy of
                    # [8, 1024]: halves the per-op copy overhead
                    cps = cwpool.tile([2 * BL, 1024], F32, tag="cw")
                    for j in range(2):
                        ti = g * HTG + 2 * k2 + j
                        nc.tensor.matmul(
                            cps[:, j * 512:(j + 1) * 512], lhsT=fused_sb[:],
                            rhs=hwt_sb[:, ti * 512:(ti + 1) * 512],
                            start=True, stop=True,
                        )
                    copy_rot(k2, cw_sb[:, k2 * 1024:(k2 + 1) * 1024], cps[:])
                if g == 0:
                    fetch_alloc()
                slice_s0(g, cw_sb)
                # bounce for samples 1-3 via DRAM scratch; on the ACT DGE so
                # the Sync queue stays clear for sample 0's slices
                for par in range(2):
                    nc.scalar.dma_start(
                        out=cw_dram[1:, g * 12288 + par * 6144:
                                    g * 12288 + (par + 1) * 6144],
                        in_=cw_sb[par * BL + 1:(par + 1) * BL, :],
                    )
            # high_priority pins these ahead of the up-phase output DMAs in
            # the scheduler's per-engine ordering — otherwise the in-order
            # Sync queue head-of-line-blocks the fetches behind out tiles
            with tc.high_priority():
                fetch_sample(1, nc.scalar)
                fetch_sample(2, nc.sync)
                fetch_sample(3, nc.sync)
            # keep the PE clock up while the fetches land
            warm2 = cwpool.tile([2 * BL, 1024], F32, tag="cw", name="warm2")
            for _ in range(4):
                nc.tensor.matmul(
                    warm2[:, 0:512], lhsT=fused_sb[:], rhs=hwt_sb[:, 0:512],
                    start=True, stop=True, skip_group_check=True,
                )

        # ---------- phase C: conv + up-projection ----------
        with tc.tile_pool(name="cvps", bufs=2, space="PSUM") as cvpool, \
             tc.tile_pool(name="upps", bufs=3, space="PSUM") as uppool:

            def conv_prep(b):
                nc.vector.tensor_add(cwp_t[b][:], cwp_t[b][:], hbp2_sb[:])
                nc.vector.tensor_add(
                    cws_t[b][0:64, :], cws_t[b][0:64, :], hbp3_sb[:]
                )

            def conv_half(b, hc):
                cwp_sb, cws_sb = cwp_t[b], cws_t[b]
                cvp = cvpool.tile([64, 420], F32, tag="cv")
                off = b * (HP * WP) + hc * 14 * WP
                for dh in range(3):
                    o = off + dh * WP
                    nc.tensor.matmul(
                        cvp[:, 0:NF],
                        lhsT=cwp_sb[:, dh * 64:(dh + 1) * 64],
                        rhs=s1f[:, o:o + NF],
                        start=(dh == 0), stop=False,
                    )
                    nc.tensor.matmul(
                        cvp[:, 0:NF],
                        lhsT=cws_sb[:, dh * 64:(dh + 1) * 64],
                        rhs=s1f[:, o + 2:o + 2 + NF],
                        start=False, stop=(dh == 2),
                    )
                oy = b * 784 + hc * NB
                nc.scalar.activation(
                    yg_sb[0:64, oy:oy + NB].rearrange(
                        "p (h w) -> p h w", h=14, w=W
                    ),
                    cvp[:].rearrange("p (h w) -> p h w", h=14, w=WP)[:, :, 0:W],
                    AF.Gelu_apprx_sigmoid,
                )

            osb_t = [None]

            def up_tile(t):
                r0 = t * 128
                m = min(128, R - r0)
                half = t % 2
                if half == 0:
                    osb_t[0] = outpool.tile([128, 2 * C], BF, tag="osb", name="osb")
                osb = osb_t[0]
                upp = uppool.tile([128, 768], F32, tag="up", name="upp")
                nc.tensor.matmul(
                    upp[:m, 0:512],
                    lhsT=yg_sb[:, r0:r0 + m],
                    rhs=upw_sb[:, 0:512],
                    start=True, stop=True,
                )
                nc.tensor.matmul(
                    upp[:m, 512:768],
                    lhsT=yg_sb[:, r0:r0 + m],
                    rhs=upw_sb[:, 512:768],
                    start=True, stop=True,
                )
                # split 512/256 across DVE+ACT: ACT is the busier engine in
                # this phase (conv qgelus), so it gets the smaller piece;
                # tiles right after a conv burst go fully to DVE so they
                # aren't stuck behind the queued qgelus on ACT
                o0 = half * C
                if t in (0, 1, 3, 6, 9, 12):
                    nc.vector.tensor_copy(out=osb[:m, o0:o0 + C], in_=upp[:m, :])
                elif t >= 22:
                    # end-game: balance the engines so the final copies and
                    # DMAs drain with minimum latency
                    nc.vector.tensor_copy(
                        out=osb[:m, o0:o0 + 384], in_=upp[:m, 0:384]
                    )
                    nc.scalar.activation(
                        osb[:m, o0 + 384:o0 + C], upp[:m, 384:768], AF.Copy
                    )
                else:
                    nc.vector.tensor_copy(
                        out=osb[:m, o0:o0 + 512], in_=upp[:m, 0:512]
                    )
                    nc.scalar.activation(
                        osb[:m, o0 + 512:o0 + C], upp[:m, 512:768], AF.Copy
                    )
                if t >= 22:
                    nc.sync.dma_start(out=out[r0:r0 + m, :], in_=osb[:m, o0:o0 + C])
                elif half == 1:
                    p0 = (t - 1) * 128
                    nc.sync.dma_start(
                        out=out[p0:p0 + 256, :].rearrange("(t p) c -> p t c", p=128),
                        in_=osb[:].rearrange("p (t c) -> p t c", t=2),
                    )

            # samples 0/1 convolve up front (up tiles 0-11 read their
            # rows); samples 2/3 interleave one conv HALF per 3 up tiles so
            # the ACT queue never bunches qgelus ahead of up-tile copies
            conv_prep(0)
            conv_half(0, 0); conv_half(0, 1)
            conv_prep(1)
            conv_half(1, 0); conv_half(1, 1)
            conv_prep(2)
            conv_prep(3)
            up_tile(0); up_tile(1); up_tile(2)
            conv_half(2, 0)
            up_tile(3); up_tile(4); up_tile(5)
            conv_half(2, 1)
            up_tile(6); up_tile(7); up_tile(8)
            conv_half(3, 0)
            up_tile(9); up_tile(10); up_tile(11)
            conv_half(3, 1)
            for t in range(12, NUT):
                up_tile(t)

    nc.compile()
    return nc


def _prep_host(inputs):
    f = lambda a: np.ascontiguousarray(np.asarray(a, dtype=np.float32))
    x = f(inputs["x"])
    meta_w1, meta_b1 = f(inputs["meta_w1"]), f(inputs["meta_b1"])
    meta_w2, meta_b2 = f(inputs["meta_w2"]), f(inputs["meta_b2"])
    layer_emb = f(inputs["layer_emb"])
    hyper_w, hyper_b = f(inputs["hyper_w"]), f(inputs["hyper_b"])
    down_w, down_b = f(inputs["down_w"]), f(inputs["down_b"])
    up_w, up_b = f(inputs["up_w"]), f(inputs["up_b"])

    # SBUF-layout stationary: wstk[p, kt*128 + m] = W[kt*128 + p, m]
    # (W = [meta_w1; down_w].T, [C, 128]) so the DMA is a flat [128, 1536B]
    wstk = np.ascontiguousarray(
        np.concatenate([meta_w1, down_w], axis=0).T
        .reshape(6, 128, 128).transpose(1, 0, 2)
    ).reshape(128, C).astype(BF16)
    csm = np.ascontiguousarray(
        np.stack([meta_b1, down_b, meta_b2 + layer_emb], axis=1)
    )  # [64, 3]
    w2t = np.ascontiguousarray(meta_w2.T / 784.0).astype(BF16)  # w2[p,o]/HW

    # hyper_w [j, e], j = (do, di, kh, kw)  ->  HWTperm [e, j'], j' = (t, di, do)
    hw5 = hyper_w.reshape(D, D, 3, 3, EMB)            # do, di, kh, kw, e
    hwtp = np.ascontiguousarray(hw5.transpose(4, 2, 3, 1, 0)).reshape(EMB, JTOT)
    # within each 24-chunk group, pack chunks (24g+k, 24g+12+k) on the two
    # K-halves of tile (g, k): each PSUM parity row then holds a contiguous
    # 6144-element j-range per group -> [128, NHT*512]
    hwt = np.ascontiguousarray(
        hwtp.reshape(EMB, 3, 2, HTG, 512).transpose(2, 0, 1, 3, 4)
    ).reshape(128, NHT * 512).astype(BF16)
    # hyper bias in the conv-weight tile layouts
    hb4 = hyper_b.reshape(D, D, 3, 3).transpose(3, 1, 2, 0)  # [dw, di, dh, do]
    hbp2 = np.ascontiguousarray(hb4[0:2]).reshape(128, 192).astype(BF16)
    hbp3 = np.ascontiguousarray(hb4[2]).reshape(64, 192).astype(BF16)

    upw = np.ascontiguousarray(
        np.concatenate([up_w.T, up_b.reshape(1, C)], axis=0)
    ).astype(BF16)  # [65, C]

    shared = dict(wstk=wstk, csm=csm, w2t=w2t,
                  hwt=hwt, hbp2=hbp2, hbp3=hbp3, upw=upw)
    in_maps = []
    for k in range(NCORES):
        m = dict(shared)
        xc = x[k * BL:(k + 1) * BL].reshape(R, C)
        # partition-major chunk layout: xtp[p, q*2352 + kt*392 + n] =
        # xc[q*392 + n, kt*128 + p]; per-partition rows are contiguous so
        # chunk-pair DMAs use 9408B descriptors
        xtp = np.ascontiguousarray(
            xc.reshape(NQ, NB, 6, 128).transpose(3, 0, 2, 1)
        ).reshape(128, NQ * 6 * NB).astype(BF16)
        m["xtp"] = xtp
        in_maps.append(m)
    return in_maps


def kernel(**inputs) -> np.ndarray:
    if "nc" not in _cached:
        _cached["nc"] = _build_program()
    nc = _cached["nc"]
    in_maps = _prep_host(inputs)
    res = run_bass_kernel_spmd(nc, in_maps, list(range(NCORES)), trace=TRACE)
    global LAST_EXEC_NS
    if TRACE and res.exec_time_ns is not None:
        LAST_EXEC_NS = res.exec_time_ns
        print(f"HW exec time: {res.exec_time_ns} ns")
    outs = [
        res.results[k]["out"].astype(np.float32).reshape(BL, H, W, C)
        for k in range(NCORES)
    ]
    return np.concatenate(outs, axis=0)


# revision 75
# speedup vs baseline: 1.0114x; 1.0114x over previous
"""Trainium2 Bass kernel for the Convpass-swin hypernet-fuse adapter module.

Data-parallel over batch: 32 samples -> 8 cores x 4 samples; small weights
replicated. All heavy matmuls run in bf16 (fp32 PSUM accumulate); the
tolerance budget (2e-2) dwarfs bf16 rounding (~2e-3).

Per-core dataflow (R = 4*28*28 = 3136 rows, C=768, D=EMB=64):
  1. x is transposed and bf16-cast on the host into chunk-major layout
     xtp[q*128+p, kt*392+n] (8 chunks of 392 rows). DMA order: wstk, all 8
     x chunks (just-in-time for the stacked matmuls), then hwt in 6 pieces
     so the hypernet matmuls stream right behind their own DMA.
  2. Per chunk: 6 K-tile matmuls -> PSUM [128, 392]; rows 0:64 ACT
     Relu(+b1, accum_out) for the meta path, rows 64:128 ACT
     Gelu_apprx_sigmoid(+down_b) = exact qgelu -> bf16 s1 in a zero-padded
     [128, 4, 30, 30] buffer; DVE duplicates it one column shifted on
     partitions 64:128 so conv taps (dh,0),(dh,1) fuse into K=128 matmuls.
  3. prompt accumulates in PSUM via 2 matmuls over mha column halves
     (w2.T/784 stationary); fused = prompt + b2 + layer_emb written twice
     into a tiny [128, 8] block-diagonal stationary (two parity chunks).
  4. Hypernet: one K=128 bf16 matmul per [128,512] weight tile -> PSUM
     [8, 512]; copies (DVE/ACT/GpSimd rotating) -> bf16 staging [8, 6144]
     per 12-tile group, bounced via DRAM scratch into conv-weight layout.
     Fetches for samples 0/1 are split by dh and prefetched as soon as the
     covering groups are written.
  5. Conv per (sample, 14-row half): 3 paired K=128 + 3 single K=64 bf16
     matmuls stream FLAT N=418 windows of the padded s1 (garbage lands in
     pad columns of PSUM [64, 420]); ACT qgelu reads the valid columns
     strided -> yg bf16 [65, R] (row 64 = ones).
  6. Up-projection in 25 M=128 row tiles across the whole yg (tiles may
     span samples): 2 matmuls (N=512+256) -> PSUM [128, 768], one rotating
     copy -> bf16 staging, output DMA per 2 tiles.
"""

import sys

sys.path.insert(0, "/opt/trn_rl_repo")

import ml_dtypes
import numpy as np

import concourse.bass as bass
import concourse.tile as tile
from concourse import bacc, mybir
from concourse.bass_utils import run_bass_kernel_spmd

F32 = mybir.dt.float32
BF = mybir.dt.bfloat16
AF = mybir.ActivationFunctionType
OP = mybir.AluOpType
BF16 = ml_dtypes.bfloat16

B, H, W, C, D, EMB = 32, 28, 28, 768, 64, 64
NCORES = 8
BL = B // NCORES            # samples per core
R = BL * H * W              # 3136 rows per core
HP, WP = H + 2, W + 2       # padded 30x30
JTOT = D * D * 9            # 36864 hypernet outputs per sample
NCH = JTOT // 512           # 72 chunks of 512
NHT = NCH // 2              # 36 hypernet weight tiles [128, 512]
HTG = 12                    # hypernet tiles per group (= one kh tap row)
NHWP = 6                    # hwt DMA pieces (6 tiles each)
NB = 392                    # half-sample chunk (14 rows of 28)
NQ = R // NB                # 8 stacked chunks per core
NF = 418                    # flat conv window length (13*30 + 28)
NUT = (R + 127) // 128      # 25 up-projection row tiles

TRACE = False               # set True (e.g. from test.py) to capture a profile
LAST_EXEC_NS = None         # filled from the profile when TRACE is on

_cached = {}

# f32 whose bit pattern is two bf16 1.0s, for memsets on bf16 tiles
_ONES_BF16_PAIR = float(
    np.frombuffer(np.array([0x3F803F80], dtype=np.uint32).tobytes(), dtype=np.float32)[0]
)


def _build_program():
    nc = bacc.Bacc("TRN2", target_bir_lowering=False, debug=False)

    xtp = nc.declare_dram_parameter("xtp", [128, NQ * 6 * NB], BF, isOutput=False).ap()
    wstk = nc.declare_dram_parameter("wstk", [128, C], BF, isOutput=False).ap()
    # packed small biases: col 0 = meta_b1, col 1 = down_b, col 2 = b2+emb
    csm = nc.declare_dram_parameter("csm", [64, 3], F32, isOutput=False).ap()
    w2t = nc.declare_dram_parameter("w2t", [64, 64], BF, isOutput=False).ap()
    hwt = nc.declare_dram_parameter("hwt", [128, NHT * 512], BF, isOutput=False).ap()
    hbp2 = nc.declare_dram_parameter("hbp2", [128, 192], BF, isOutput=False).ap()
    hbp3 = nc.declare_dram_parameter("hbp3", [64, 192], BF, isOutput=False).ap()
    upw = nc.declare_dram_parameter("upw", [65, C], BF, isOutput=False).ap()
    out = nc.declare_dram_parameter("out", [R, C], BF, isOutput=True).ap()

    with tile.TileContext(nc) as tc, \
         tc.tile_pool(name="consts", bufs=1) as cpool, \
         tc.tile_pool(name="xin", bufs=3) as xinpool, \
         tc.tile_pool(name="work", bufs=2) as wpool, \
         tc.tile_pool(name="cwsb", bufs=3) as cwsbpool, \
         tc.tile_pool(name="cwtp", bufs=4) as cwtpool, \
         tc.tile_pool(name="outp", bufs=2) as outpool, \
         tc.tile_pool(name="dram", bufs=1, space="DRAM") as dpool:

        # ---------- constants / standing buffers ----------
        # DMA issue order is the priority order on the Sync queue: wstk
        # first (every chunk matmul needs it), then the x chunks stream
        # just-in-time inside the loop, then hwt, then later-phase consts.
        wstk_sb = cpool.tile([128, 768], BF, tag="wstk")
        nc.sync.dma_start(out=wstk_sb[:], in_=wstk)

        s1pad = cpool.tile([128, BL * HP * WP], BF, tag="s1pad")
        nc.gpsimd.memset(s1pad[:].bitcast(F32), 0.0)
        mha_sb = cpool.tile([64, NQ], F32, tag="mha")
        fused_sb = cpool.tile([128, 2 * BL], BF, tag="fused")
        yg_sb = cpool.tile([65, R], BF, tag="yg")
        nc.vector.memset(yg_sb[64:65, :].bitcast(F32), _ONES_BF16_PAIR)
        hwt_sb = cpool.tile([128, NHT * 512], BF, tag="hwt")

        # preload both ACT function tables during the DMA-bound window so
        # the first real Relu / qgelu doesn't eat a ~1.3us table load
        ascr = cpool.tile([64, 1], F32, tag="ascr")
        nc.vector.memset(ascr[:], 0.0)
        ascr2 = cpool.tile([64, 1], F32, tag="ascr2")
        nc.scalar.activation(ascr2[:], ascr[:], AF.Relu)
        nc.scalar.activation(ascr2[:], ascr[:], AF.Gelu_apprx_sigmoid)

        s1v = s1pad[:].rearrange("p (b h w) -> p b h w", b=BL, h=HP, w=WP)
        s1f = s1pad[:]                     # flat [128, BL*900] view for conv

        # ---------- phase A: stacked meta1+down over 8 chunks, prompt ----------
        with tc.tile_pool(name="stkps", bufs=5, space="PSUM") as stkpool, \
             tc.tile_pool(name="auxps", bufs=1, space="PSUM") as auxpool:
            # PE clock warms up over ~5us of activity; burn the DMA-wait
            # window on junk matmuls so the first real chunk streams at
            # full rate instead of ~60% clock
            warm = auxpool.tile([128, 512], F32, tag="warm", name="warm")
            for _ in range(6):
                nc.tensor.matmul(
                    warm[:], lhsT=wstk_sb[:, 0:128], rhs=wstk_sb[:, 0:512],
                    start=True, stop=True, skip_group_check=True,
                )

            xq2 = None
            u = 0
            for q in range(NQ):
                b, hc = divmod(q, 2)
                if q < 2:
                    # first two chunks individually: lowest time-to-first-
                    # matmul while the queues are still cold
                    u = 0
                    xq2 = xinpool.tile([128, 6 * NB], BF, tag="xq1")
                    nc.sync.dma_start(
                        out=xq2[:], in_=xtp[:, q * 2352:(q + 1) * 2352]
                    )
                elif q % 2 == 0:
                    # then chunk pairs: 9408B/partition descriptors run
                    # ~30% faster per queue than single-chunk 4704B ones
                    u = 0
                    xq2 = xinpool.tile([128, 2 * 6 * NB], BF, tag="xq")
                    nc.sync.dma_start(
                        out=xq2[:],
                        in_=xtp[:, q * 2352:(q + 2) * 2352],
                    )
                else:
                    u = 1
                if q == 0:
                    csm_sb = cpool.tile([64, 3], F32, tag="csm")
                    nc.sync.dma_start(out=csm_sb[:], in_=csm)
                    brelu_sb = csm_sb[:, 0:1]
                    dwb_sb = csm_sb[:, 1:2]
                    fb_sb = csm_sb[:, 2:3]
                ps = stkpool.tile([128, NB], F32, tag="stk", name="ps")
                for kt in range(6):
                    nc.tensor.matmul(
                        ps[:],
                        lhsT=wstk_sb[:, kt * 128:(kt + 1) * 128],
                        rhs=xq2[:, u * 2352 + kt * NB:u * 2352 + (kt + 1) * NB],
                        start=(kt == 0),
                        stop=(kt == 5),
                    )
                hsc = wpool.tile([64, NB], BF, tag="hsc", name="hsc")
                ps3 = ps[64:128, :].rearrange("p (h w) -> p h w", h=14, w=W)
                h0 = hc * 14 + 1

                def relu_act():
                    nc.scalar.activation(
                        hsc[:], ps[0:64, :], AF.Relu,
                        bias=brelu_sb,
                    )

                def gelu_act():
                    nc.scalar.activation(
                        s1v[0:64, b, h0:h0 + 14, 1:W + 1], ps3,
                        AF.Gelu_apprx_sigmoid, bias=dwb_sb,
                    )

                # row-sums for the prompt go through DVE (tensor_reduce) to
                # keep the ACT engine at 2 ops/chunk, matching the PE rate
                # (fp32 out: a bf16 reduce also ACCUMULATES in bf16, which
                # destroys 392-term sums)

                def red_op():
                    nc.vector.tensor_reduce(
                        out=mha_sb[:, q:q + 1], in_=hsc[:],
                        axis=mybir.AxisListType.X, op=OP.add,
                    )

                def shift_op():
                    nc.vector.tensor_copy(
                        out=s1v[64:128, b, h0:h0 + 14, 0:W],
                        in_=s1v[0:64, b, h0:h0 + 14, 1:W + 1],
                    )

                # last chunk: gelu first on ACT / shift first on DVE, so
                # the prompt chain (gelu-independent) finishes earliest
                relu_act(); gelu_act(); red_op(); shift_op()

            # w2t is small and needed at the prompt (~25us); issue it ahead
            # of the big hypernet stream
            w2t_sb = cpool.tile([64, 64], BF, tag="w2t")
            nc.sync.dma_start(out=w2t_sb[:], in_=w2t)
            # hypernet weights stream right after the x chunks; the hyper
            # matmuls chase the pieces as they land
            npc = NHT * 512 // NHWP
            for i in range(NHWP):
                nc.sync.dma_start(
                    out=hwt_sb[:, i * npc:(i + 1) * npc],
                    in_=hwt[:, i * npc:(i + 1) * npc],
                )
            # consts for the conv/up phases ride behind
            upw_sb = cpool.tile([65, C], BF, tag="upw")
            nc.sync.dma_start(out=upw_sb[:], in_=upw)
            hbp2_sb = cpool.tile([128, 192], BF, tag="hbp2")
            nc.sync.dma_start(out=hbp2_sb[:], in_=hbp2)
            hbp3_sb = cpool.tile([64, 192], BF, tag="hbp3")
            nc.sync.dma_start(out=hbp3_sb[:], in_=hbp3)

            # prompt: pp[o, b] = sum_q w2t[:, o] . mha[:, (b, hc)], in bf16
            # (fp32 matmuls take 4 PE passes and a slow weight load)
            mhab = cpool.tile([64, NQ], BF, tag="mhab")
            nc.vector.tensor_copy(out=mhab[:], in_=mha_sb[:])
            mhv = mhab[:].rearrange("p (b h) -> p b h", b=BL)
            pp = auxpool.tile([64, BL], F32, tag="pp", name="pp")
            nc.tensor.matmul(
                pp[:], lhsT=w2t_sb[:], rhs=mhv[:, :, 0], start=True, stop=False,
            )
            nc.tensor.matmul(
                pp[:], lhsT=w2t_sb[:], rhs=mhv[:, :, 1], start=False, stop=True,
            )
            # block-diagonal [128, 8] stationary: rows 0:64 x cols 0:4 for
            # the even parity chunk, rows 64:128 x cols 4:8 for the odd
            nc.vector.memset(fused_sb[:].bitcast(F32), 0.0)
            nc.scalar.activation(fused_sb[0:64, 0:BL], pp[:], AF.Identity, bias=fb_sb)
            nc.scalar.activation(
                fused_sb[64:128, BL:2 * BL], pp[:], AF.Identity, bias=fb_sb
            )

        # ---------- phase B: hypernet ----------
        def copy_rot(i, out_ap, in_ap):
            # gpsimd cannot read PSUM on TRN2, so rotate over DVE and ACT
            if i % 2 == 0:
                nc.vector.tensor_copy(out=out_ap, in_=in_ap)
            else:
                nc.scalar.activation(out_ap, in_ap, AF.Copy)

        # host packs tile k of group g with chunks (24g+k, 24g+12+k) on the
        # two K-halves, so each cw_sb partition row is a CONTIGUOUS 6144-
        # element j-range: the group write is one DMA of 12KB descriptors
        cw_dram = dpool.tile([BL, JTOT], BF, tag="cw")
        # conv weight fetch view: j = (dh, (dw, di), do)
        cwt4 = cw_dram[:].rearrange(
            "b (dh dwdi do) -> b dwdi dh do", dh=3, dwdi=3 * D, do=D
        )
        cwp_t, cws_t = {}, {}

        def fetch_alloc():
            # single-tap (dw=2) weights live in rows 0:64 of a [128, 192]
            # tile whose upper half is zeroed, so every conv matmul runs
            # K=128 with identical PE tile_size (no reconfig stalls)
            for b in range(BL):
                cwp_t[b] = cwtpool.tile([128, 192], BF, tag="cwp", name="cwp")
                cws_t[b] = cwtpool.tile([128, 192], BF, tag="cws", name="cws")
                # DVE, not gpsimd: gpsimd op dispatch is ~us-slow and these
                # zeroed halves gate the conv single-tap matmuls
                nc.vector.memset(cws_t[b][64:128, :].bitcast(F32), 0.0)

        def fetch_sample(b, eng):
            eng.dma_start(
                out=cwp_t[b][:].rearrange("p (dh do) -> p dh do", dh=3),
                in_=cwt4[b, 0:128],
            )
            eng.dma_start(
                out=cws_t[b][0:64, :].rearrange("p (dh do) -> p dh do", dh=3),
                in_=cwt4[b, 128:192],
            )

        def slice_s0(g, cw_sb):
            # sample 0's conv stationaries gather SBUF->SBUF per group
            # (group g = tap row dh=g), skipping the DRAM bounce entirely:
            # parity row 0 holds kwdi 0:96, row 1 holds kwdi 96:192
            b = 0
            row0 = cw_sb[b:b + 1, :].rearrange("q (p do) -> q p do", do=64)
            row1 = cw_sb[BL + b:BL + b + 1, :].rearrange(
                "q (p do) -> q p do", do=64
            )
            nc.sync.dma_start(
                out=cwp_t[b][0:96, g * 64:(g + 1) * 64], in_=row0
            )
            nc.sync.dma_start(
                out=cwp_t[b][96:128, g * 64:(g + 1) * 64], in_=row1[:, 0:32]
            )
            nc.sync.dma_start(
                out=cws_t[b][0:64, g * 64:(g + 1) * 64], in_=row1[:, 32:96]
            )

        with tc.tile_pool(name="cwps", bufs=3, space="PSUM") as cwpool:
            for g in range(NHT // HTG):
                cw_sb = cwsbpool.tile([2 * BL, HTG * 512], BF, tag="cwsb")
                for k2 in range(HTG // 2):
                    # two matmul outputs per 2-bank PSUM tile, ONE copy of
                    # [8, 1024]: halves the per-op copy overhead
                    cps = cwpool.tile([2 * BL, 1024], F32, tag="cw")
                    for j in range(2):
                        ti = g * HTG + 2 * k2 + j
                        nc.tensor.matmul(
                            cps[:, j * 512:(j + 1) * 512], lhsT=fused_sb[:],
                            rhs=hwt_sb[:, ti * 512:(ti + 1) * 512],
                            start=True, stop=True,
                        )
                    copy_rot(k2, cw_sb[:, k2 * 1024:(k2 + 1) * 1024], cps[:])
                if g == 0:
                    fetch_alloc()
                slice_s0(g, cw_sb)
                # bounce for samples 1-3 via DRAM scratch; the final group's
                # writes go to the ACT DGE so the Sync queue stays clear
                # for sample 0's critical slices
                weng = nc.scalar if g == 2 else nc.sync
                for par in range(2):
                    weng.dma_start(
                        out=cw_dram[1:, g * 12288 + par * 6144:
                                    g * 12288 + (par + 1) * 6144],
                        in_=cw_sb[par * BL + 1:(par + 1) * BL, :],
                    )
            # high_priority pins these ahead of the up-phase output DMAs in
            # the scheduler's per-engine ordering — otherwise the in-order
            # Sync queue head-of-line-blocks the fetches behind out tiles
            with tc.high_priority():
                fetch_sample(1, nc.scalar)
                fetch_sample(2, nc.sync)
                fetch_sample(3, nc.sync)
            # keep the PE clock up while the fetches land
            warm2 = cwpool.tile([2 * BL, 1024], F32, tag="cw", name="warm2")
            for _ in range(4):
                nc.tensor.matmul(
                    warm2[:, 0:512], lhsT=fused_sb[:], rhs=hwt_sb[:, 0:512],
                    start=True, stop=True, skip_group_check=True,
                )

        # ---------- phase C: conv + up-projection ----------
        with tc.tile_pool(name="cvps", bufs=2, space="PSUM") as cvpool, \
             tc.tile_pool(name="upps", bufs=3, space="PSUM") as uppool:

            def conv_prep(b):
                nc.vector.tensor_add(cwp_t[b][:], cwp_t[b][:], hbp2_sb[:])
                nc.vector.tensor_add(
                    cws_t[b][0:64, :], cws_t[b][0:64, :], hbp3_sb[:]
                )

            def conv_half(b, hc):
                cwp_sb, cws_sb = cwp_t[b], cws_t[b]
                cvp = cvpool.tile([64, 420], F32, tag="cv")
                off = b * (HP * WP) + hc * 14 * WP
                for dh in range(3):
                    o = off + dh * WP
                    nc.tensor.matmul(
                        cvp[:, 0:NF],
                        lhsT=cwp_sb[:, dh * 64:(dh + 1) * 64],
                        rhs=s1f[:, o:o + NF],
                        start=(dh == 0), stop=False,
                    )
                    nc.tensor.matmul(
                        cvp[:, 0:NF],
                        lhsT=cws_sb[:, dh * 64:(dh + 1) * 64],
                        rhs=s1f[:, o + 2:o + 2 + NF],
                        start=False, stop=(dh == 2),
                    )
                oy = b * 784 + hc * NB
                nc.scalar.activation(
                    yg_sb[0:64, oy:oy + NB].rearrange(
                        "p (h w) -> p h w", h=14, w=W
                    ),
                    cvp[:].rearrange("p (h w) -> p h w", h=14, w=WP)[:, :, 0:W],
                    AF.Gelu_apprx_sigmoid,
                )

            osb_t = [None]

            def up_tile(t):
                r0 = t * 128
                m = min(128, R - r0)
                half = t % 2
                if half == 0:
                    osb_t[0] = outpool.tile([128, 2 * C], BF, tag="osb", name="osb")
                osb = osb_t[0]
                upp = uppool.tile([128, 768], F32, tag="up", name="upp")
                nc.tensor.matmul(
                    upp[:m, 0:512],
                    lhsT=yg_sb[:, r0:r0 + m],
                    rhs=upw_sb[:, 0:512],
                    start=True, stop=True,
                )
                nc.tensor.matmul(
                    upp[:m, 512:768],
                    lhsT=yg_sb[:, r0:r0 + m],
                    rhs=upw_sb[:, 512:768],
                    start=True, stop=True,
                )
                # split 512/256 across DVE+ACT: ACT is the busier engine in
                # this phase (conv qgelus), so it gets the smaller piece;
                # tiles right after a conv burst go fully to DVE so they
                # aren't stuck behind the queued qgelus on ACT
                o0 = half * C
                if t in (0, 1, 3, 6, 9, 12):
                    nc.vector.tensor_copy(out=osb[:m, o0:o0 + C], in_=upp[:m, :])
                elif t >= 22:
                    # end-game: balance the engines so the final copies and
                    # DMAs drain with minimum latency
                    nc.vector.tensor_copy(
                        out=osb[:m, o0:o0 + 384], in_=upp[:m, 0:384]
                    )
                    nc.scalar.activation(
                        osb[:m, o0 + 384:o0 + C], upp[:m, 384:768], AF.Copy
                    )
                else:
                    nc.vector.tensor_copy(
                        out=osb[:m, o0:o0 + 512], in_=upp[:m, 0:512]
                    )
                    nc.scalar.activation(
                        osb[:m, o0 + 512:o0 + C], upp[:m, 512:768], AF.Copy
                    )
                if t >= 22:
                    nc.sync.dma_start(out=out[r0:r0 + m, :], in_=osb[:m, o0:o0 + C])
                elif half == 1:
                    p0 = (t - 1) * 128
                    nc.sync.dma_start(
                        out=out[p0:p0 + 256, :].rearrange("(t p) c -> p t c", p=128),
                        in_=osb[:].rearrange("p (t c) -> p t c", t=2),
                    )

            # samples 0/1 convolve up front (up tiles 0-11 read their
            # rows); samples 2/3 interleave one conv HALF per 3 up tiles so
            # the ACT queue never bunches qgelus ahead of up-tile copies
            conv_prep(0)
            conv_half(0, 0); conv_half(0, 1)
            conv_prep(1)
            conv_half(1, 0); conv_half(1, 1)
            conv_prep(2)
            conv_prep(3)
            up_tile(0); up_tile(1); up_tile(2)
            conv_half(2, 0)
            up_tile(3); up_tile(4); up_tile(5)
            conv_half(2, 1)
            up_tile(6); up_tile(7); up_tile(8)
            conv_half(3, 0)
            up_tile(9); up_tile(10); up_tile(11)
            conv_half(3, 1)
            for t in range(12, NUT):
                up_tile(t)

    nc.compile()
    return nc


def _prep_host(inputs):
    f = lambda a: np.ascontiguousarray(np.asarray(a, dtype=np.float32))
    x = f(inputs["x"])
    meta_w1, meta_b1 = f(inputs["meta_w1"]), f(inputs["meta_b1"])
    meta_w2, meta_b2 = f(inputs["meta_w2"]), f(inputs["meta_b2"])
    layer_emb = f(inputs["layer_emb"])
    hyper_w, hyper_b = f(inputs["hyper_w"]), f(inputs["hyper_b"])
    down_w, down_b = f(inputs["down_w"]), f(inputs["down_b"])
    up_w, up_b = f(inputs["up_w"]), f(inputs["up_b"])

    # SBUF-layout stationary: wstk[p, kt*128 + m] = W[kt*128 + p, m]
    # (W = [meta_w1; down_w].T, [C, 128]) so the DMA is a flat [128, 1536B]
    wstk = np.ascontiguousarray(
        np.concatenate([meta_w1, down_w], axis=0).T
        .reshape(6, 128, 128).transpose(1, 0, 2)
    ).reshape(128, C).astype(BF16)
    csm = np.ascontiguousarray(
        np.stack([meta_b1, down_b, meta_b2 + layer_emb], axis=1)
    )  # [64, 3]
    w2t = np.ascontiguousarray(meta_w2.T / 784.0).astype(BF16)  # w2[p,o]/HW

    # hyper_w [j, e], j = (do, di, kh, kw)  ->  HWTperm [e, j'], j' = (t, di, do)
    hw5 = hyper_w.reshape(D, D, 3, 3, EMB)            # do, di, kh, kw, e
    hwtp = np.ascontiguousarray(hw5.transpose(4, 2, 3, 1, 0)).reshape(EMB, JTOT)
    # within each 24-chunk group, pack chunks (24g+k, 24g+12+k) on the two
    # K-halves of tile (g, k): each PSUM parity row then holds a contiguous
    # 6144-element j-range per group -> [128, NHT*512]
    hwt = np.ascontiguousarray(
        hwtp.reshape(EMB, 3, 2, HTG, 512).transpose(2, 0, 1, 3, 4)
    ).reshape(128, NHT * 512).astype(BF16)
    # hyper bias in the conv-weight tile layouts
    hb4 = hyper_b.reshape(D, D, 3, 3).transpose(3, 1, 2, 0)  # [dw, di, dh, do]
    hbp2 = np.ascontiguousarray(hb4[0:2]).reshape(128, 192).astype(BF16)
    hbp3 = np.ascontiguousarray(hb4[2]).reshape(64, 192).astype(BF16)

    upw = np.ascontiguousarray(
        np.concatenate([up_w.T, up_b.reshape(1, C)], axis=0)
    ).astype(BF16)  # [65, C]

    shared = dict(wstk=wstk, csm=csm, w2t=w2t,
                  hwt=hwt, hbp2=hbp2, hbp3=hbp3, upw=upw)
    in_maps = []
    for k in range(NCORES):
        m = dict(shared)
        xc = x[k * BL:(k + 1) * BL].reshape(R, C)
        # partition-major chunk layout: xtp[p, q*2352 + kt*392 + n] =
        # xc[q*392 + n, kt*128 + p]; per-partition rows are contiguous so
        # chunk-pair DMAs use 9408B descriptors
        xtp = np.ascontiguousarray(
            xc.reshape(NQ, NB, 6, 128).transpose(3, 0, 2, 1)
        ).reshape(128, NQ * 6 * NB).astype(BF16)
        m["xtp"] = xtp
        in_maps.append(m)
    return in_maps


def kernel(**inputs) -> np.ndarray:
    if "nc" not in _cached:
        _cached["nc"] = _build_program()
    nc = _cached["nc"]
    in_maps = _prep_host(inputs)
    res = run_bass_kernel_spmd(nc, in_maps, list(range(NCORES)), trace=TRACE)
    global LAST_EXEC_NS
    if TRACE and res.exec_time_ns is not None:
        LAST_EXEC_NS = res.exec_time_ns
        print(f"HW exec time: {res.exec_time_ns} ns")
    outs = [
        res.results[k]["out"].astype(np.float32).reshape(BL, H, W, C)
        for k in range(NCORES)
    ]
    return np.concatenate(outs, axis=0)
